# revision 10
# baseline (speedup 1.0000x reference)
"""BatchAllTripletLoss on 8 Trainium2 NeuronCores via Bass/Tile.

Math: for anchors i, positives j (same label, j!=i), negatives k (diff label):
  total        = sum_{i,j,k} relu(d_ij - d_ik + margin)
  num_non_easy = #{(i,j,k): d_ik < d_ij + margin}
  loss         = total / num_non_easy ; frac = num_non_easy / num_valid

Key idea: samples are SORTED BY LABEL on the host, so each anchor's
positives j live in one contiguous window of at most 128 sorted positions.
The O(n^3) triplet work then needs only ONE [128, 640] compare tile per
anchor (j = its class window, k = all samples) instead of five:
  - masked row v'_k = d_ak + BIG*(same label), bf16, staged to DRAM and
    DMA-broadcast to [128, 4, 640] (stride-0 partition source, 4 anchors
    per transfer).
  - window thresholds t'_p = (d_{a, w_a+p} + margin) * posmask, gathered
    from the distance rows with ONE indirect DMA (per-anchor element
    offsets w_a are host data -> program is label-independent), f32,
    split hi/lo into bf16 lhsT columns [t_hi | t_lo | 1 | 1].
  - compare M[p, k] = (v'_k < t'_p): DVE is_lt bf16 (~300ns) for every
    other anchor, ACT Sign (host-corrected) for the rest.
  - PE reduces M: psum rows 32s..32s+3 (4 anchors per [128, 640] psum
    tile via explicit tile_position): t_hi*M, t_lo*M, q_k, q_k.
  - ONE drain per psum tile: DVE scalar_tensor_tensor against a weight
    tile holding [1 | 1 | dist row | 1] per anchor (+accum), giving the
    W rows, sum_k d_ak q_ak, and the count in one pass.
  total = sum(t'*M) - sum(d*q);  count = sum(q).  Host combines in f64.
num_valid is pure label counting (host, exact).
"""

import numpy as np

N = 640
D = 128
NCORES = 8
NLOC = N // NCORES            # 80 anchors per core
NT = NLOC // 4                # 20 psum tiles, 4 anchors each
MARGIN = 1.9
BIG = 1.0e9
WMAX = 128                    # class-window width (max class size)

_CACHE = {}


def _is_act(la):
    return la % 2 == 1


def _build_program():
    import concourse.bass as bass
    import concourse.bacc as bacc
    import concourse.mybir as mybir
    import concourse.tile as tile
    from concourse.masks import make_identity

    f32 = mybir.dt.float32
    bf16 = mybir.dt.bfloat16
    i32 = mybir.dt.int32
    Alu = mybir.AluOpType
    Act = mybir.ActivationFunctionType

    nc = bacc.Bacc("TRN2", target_bir_lowering=False, debug=False,
                   num_devices=NCORES)

    efT = nc.declare_dram_parameter("efT", [D, N], f32, isOutput=False)
    elocT = nc.declare_dram_parameter("elocT", [D, NLOC], f32, isOutput=False)
    labrow = nc.declare_dram_parameter("labrow", [1, N], f32, isOutput=False)
    llocT = nc.declare_dram_parameter("llocT", [NLOC, 1], f32, isOutput=False)
    woff = nc.declare_dram_parameter("woff", [NLOC, 1], i32, isOutput=False)
    posw = nc.declare_dram_parameter("posw", [WMAX, NLOC], f32, isOutput=False)
    # out cols: 0:NT fused stt sums (W rows / d*q row / count row), NT dsum
    OUTC = NT + 1
    out_d = nc.declare_dram_parameter("out", [128, OUTC], f32, isOutput=True)
    out2_d = nc.declare_dram_parameter("out2", [1, NLOC], f32, isOutput=True)

    from contextlib import ExitStack
    with tile.TileContext(nc) as tc:
        with (
            tc.tile_pool(name="singles", bufs=1) as sg,
            tc.tile_pool(name="vbp", bufs=5) as vbp,
            tc.tile_pool(name="mtp", bufs=10) as mtp,
            tc.tile_pool(name="drs", bufs=2) as drs,
            tc.tile_pool(name="dram", bufs=1, space="DRAM") as dram,
        ):
            pro_stack = ExitStack()
            ps_mm = pro_stack.enter_context(
                tc.tile_pool(name="ps_mm", bufs=1, space="PSUM"))
            ps_tr = pro_stack.enter_context(
                tc.tile_pool(name="ps_tr", bufs=1, space="PSUM"))

            # weight tile for the fused drains: 1.0 everywhere except the
            # per-anchor dist rows (filled after DIST is computed)
            DPW = sg.tile([128, NT, N], f32)
            nc.gpsimd.memset(DPW[:], 1.0)

            # ---- load inputs (split across the two idle dispatch rings) ----
            EF = sg.tile([D, N], f32)
            nc.gpsimd.dma_start(out=EF[:, 0:320], in_=efT[:, 0:320])
            nc.sync.dma_start(out=EF[:, 320:N], in_=efT[:, 320:N])
            EL = sg.tile([D, NLOC], f32)
            nc.sync.dma_start(out=EL[:], in_=elocT[:])
            LR = sg.tile([1, N], f32)
            nc.sync.dma_start(out=LR[:], in_=labrow[:])
            LLT = sg.tile([NLOC, 1], f32)
            nc.sync.dma_start(out=LLT[:], in_=llocT[:])
            WOFF = sg.tile([NLOC, 1], i32)
            nc.sync.dma_start(out=WOFF[:], in_=woff[:])
            POSW = sg.tile([WMAX, NLOC], f32)
            nc.sync.dma_start(out=POSW[:], in_=posw[:])
            LBC = sg.tile([128, N], f32)
            nc.sync.dma_start(out=LBC[0:NLOC, :],
                              in_=labrow[:].to_broadcast([NLOC, N]))

            ident = sg.tile([128, 128], f32)
            make_identity(nc, ident[:])
            ones = sg.tile([128, 1], f32)
            nc.vector.memset(ones[:], 1.0)
            onesb = sg.tile([128, 1], bf16)
            nc.vector.memset(onesb[:], 1.0)
            onesrow = sg.tile([1, NLOC], bf16)
            nc.vector.memset(onesrow[:], 1.0)

            # ---- pairwise distance rows: d2 = sq_a + sq_k - 2 e_a.e_k ----
            # psum accumulates (-2 e_a.e_k) + broadcast sq_k; bf16 operands
            EFM = sg.tile([D, N], bf16)
            nc.vector.tensor_scalar_mul(out=EFM[:], in0=EF[:], scalar1=-2.0)
            ELB = sg.tile([D, NLOC], bf16)
            nc.vector.tensor_copy(ELB[:], EL[:])
            Esqb = sg.tile([D, N], bf16)
            nc.vector.tensor_mul(Esqb[:], EF[:], EF[:])
            ELsq = sg.tile([D, NLOC], f32)
            nc.vector.tensor_mul(ELsq[:], EL[:], EL[:])

            sqf_ps = ps_mm.tile([1, N], f32, tag="pro2", name="sqf")
            nc.tensor.matmul(sqf_ps[:, 0:512], onesb[:], Esqb[:, 0:512])
            nc.tensor.matmul(sqf_ps[:, 512:N], onesb[:], Esqb[:, 512:N])
            SQFB = sg.tile([1, N], bf16)
            nc.vector.tensor_copy(SQFB[:], sqf_ps[:])

            sql_ps = ps_tr.tile([NLOC, 1], f32, tag="pro3", name="sql")
            nc.tensor.matmul(sql_ps[:], ELsq[:], ones[:])
            SQL = sg.tile([NLOC, 1], f32)
            nc.vector.tensor_copy(SQL[:], sql_ps[:])

            dot_ps = ps_mm.tile([NLOC, N], f32, tag="pro", name="dot")
            nc.tensor.matmul(dot_ps[:, 0:512], ELB[:], EFM[:, 0:512],
                             start=True, stop=False)
            nc.tensor.matmul(dot_ps[:, 512:N], ELB[:], EFM[:, 512:N],
                             start=True, stop=False)
            nc.tensor.matmul(dot_ps[:, 0:512], onesrow[:],
                             SQFB[:, 0:512], start=False, stop=True,
                             skip_group_check=True)
            nc.tensor.matmul(dot_ps[:, 512:N], onesrow[:],
                             SQFB[:, 512:N], start=False, stop=True,
                             skip_group_check=True)

            PRE = sg.tile([NLOC, N], f32)
            nc.vector.tensor_scalar(out=PRE[:], in0=dot_ps[:], scalar1=SQL[:],
                                    scalar2=0.0, op0=Alu.add, op1=Alu.max)
            DIST = sg.tile([NLOC, N], f32)
            nc.scalar.activation(out=DIST[:], in_=PRE[:], func=Act.Sqrt)

            # masked v' rows, bf16 (fused add + cast)
            EQB = sg.tile([NLOC, N], f32)
            nc.vector.tensor_scalar(out=EQB[:], in0=LBC[0:NLOC, :], scalar1=LLT[:],
                                    scalar2=BIG, op0=Alu.is_equal, op1=Alu.mult)
            VMB = sg.tile([NLOC, N], bf16)
            nc.vector.tensor_add(VMB[:], DIST[:], EQB[:])
            vmd = dram.tile([NLOC, N], bf16)
            for ch in range(4):
                nc.sync.dma_start(out=vmd[20 * ch:20 * ch + 20, :],
                                  in_=VMB[20 * ch:20 * ch + 20, :])

            # dist rows -> DRAM (gather source), and into DPW rows 32s+2
            dist_d = dram.tile([1, NLOC * N], f32)
            dfl = dist_d[:]
            dst = bass.AP(tensor=dfl.tensor, offset=0,
                          ap=[[N, NLOC], [1, N]])
            nc.sync.dma_start(out=dst, in_=DIST[:])
            pstr = DPW[:].ap[0][0]
            for t in range(NT):
                sl = DPW[2:3, t, :]
                dpdst = bass.AP(tensor=sl.tensor, offset=sl.offset,
                                ap=[[32 * pstr, 4], [1, N]])
                nc.sync.dma_start(out=dpdst, in_=DIST[4 * t:4 * t + 4, :])

            # window thresholds via indirect gather: TQR[la, p] =
            # dist[la, w_la + p]; offsets woff = 640*la + w_la
            TQR = sg.tile([NLOC, WMAX], f32)
            nc.gpsimd.indirect_dma_start(
                out=TQR[:], out_offset=None,
                in_=dist_d[:],
                in_offset=bass.IndirectOffsetOnAxis(ap=WOFF[:, :1], axis=1),
            )
            tq_ps = ps_tr.tile([WMAX, NLOC], f32, tag="tr", name="tq")
            nc.tensor.transpose(tq_ps[:], TQR[:], ident[0:NLOC, 0:NLOC])
            TQ = sg.tile([WMAX, NLOC], f32)
            nc.vector.tensor_scalar_add(out=TQ[:], in0=tq_ps[:], scalar1=MARGIN)
            nc.vector.tensor_mul(TQ[:], TQ[:], POSW[:])

            # lhsT tiles [128, NLOC, 4] bf16: [t_hi | t_lo | 1 | 1]
            LHSB = sg.tile([WMAX, NLOC, 4], bf16)
            nc.vector.tensor_copy(LHSB[:, :, 0], TQ[:])
            thf = sg.tile([WMAX, NLOC], f32)
            nc.vector.tensor_copy(thf[:], LHSB[:, :, 0])
            nc.vector.tensor_sub(thf[:], TQ[:], thf[:])
            nc.vector.tensor_copy(LHSB[:, :, 1], thf[:])
            nc.vector.memset(LHSB[:, :, 2:4], 1.0)

            # Tsum_a = sum_p t'_ap (for ACT-sign corrections)
            ts_ps = ps_tr.tile([1, NLOC], f32, tag="tr", name="ts_ps")
            nc.tensor.matmul(ts_ps[:], ones[:], TQ[:])
            TSROW = sg.tile([1, NLOC], f32)
            nc.vector.tensor_copy(TSROW[:], ts_ps[:])
            nc.sync.dma_start(out=out2_d[:], in_=TSROW[:])

            # dist row sums (for sign-anchor corrections)
            DSC = sg.tile([NLOC, N], f32)
            DSUM = sg.tile([NLOC, 1], f32)
            nc.scalar.activation(out=DSC[:], in_=DIST[:], func=Act.Identity,
                                 bias=0.0, scale=1.0, accum_out=DSUM[:])

            pro_stack.close()
            wq_stack = ExitStack()
            ps_wq = wq_stack.enter_context(
                tc.tile_pool(name="ps_wq", bufs=3, space="PSUM"))

            SC = sg.tile([128, NT], f32)

            # ---- main loop: one [128, 640] compare + matmul pair/anchor ----
            vb4_cache = {}
            wq = None
            for la in range(NLOC):
                t, s = la // 4, la % 4
                if s == 0:
                    vb4 = vbp.tile([128, 4, N], bf16, tag="vb", name="vb")
                    sl = vmd[la:la + 4, :]
                    bsrc = bass.AP(tensor=sl.tensor, offset=sl.offset,
                                   ap=[[0, 128]] + [list(p) for p in sl.ap])
                    nc.gpsimd.dma_start(out=vb4[:], in_=bsrc)
                    vb4_cache[0] = vb4
                    wq = ps_wq.tile([128, N], f32, tag="wq", name="wq")
                vb = vb4_cache[0][:, s, :]
                mt = mtp.tile([128, N], bf16, tag="mt", name="mt")
                if _is_act(la):
                    nc.scalar.activation(out=mt[:], in_=vb[:], func=Act.Sign,
                                         bias=TQ[:, la:la + 1], scale=-1.0)
                else:
                    nc.vector.tensor_scalar(out=mt[:], in0=vb[:],
                                            scalar1=TQ[:, la:la + 1],
                                            scalar2=None, op0=Alu.is_lt)
                nc.tensor.matmul(wq[32 * s:32 * s + 4, 0:512],
                                 LHSB[:, la], mt[:, 0:512],
                                 start=True, stop=True,
                                 tile_position=(0, 32 * s))
                nc.tensor.matmul(wq[32 * s:32 * s + 4, 512:N],
                                 LHSB[:, la], mt[:, 512:N],
                                 start=True, stop=True,
                                 tile_position=(0, 32 * s))
                if s == 3:
                    sb = drs.tile([128, N], bf16, tag="sb", name="sb")
                    nc.vector.scalar_tensor_tensor(out=sb[:], in0=wq[:],
                                                   scalar=1.0,
                                                   in1=DPW[:, t, :],
                                                   op0=Alu.mult, op1=Alu.mult,
                                                   accum_out=SC[:, t:t + 1])

            # ---- stage outputs ----
            OUTS = sg.tile([128, OUTC], f32)
            nc.vector.tensor_copy(OUTS[:, 0:NT], SC[:])
            nc.vector.tensor_copy(OUTS[0:NLOC, NT:NT + 1], DSUM[:])
            nc.gpsimd.dma_start(out=out_d[:], in_=OUTS[:])
            wq_stack.close()

    nc.compile()
    return nc


def _get_program():
    if "nc" not in _CACHE:
        _CACHE["nc"] = _build_program()
    return _CACHE["nc"]


def _make_inputs(embeddings: np.ndarray, labels: np.ndarray):
    e = np.ascontiguousarray(embeddings.reshape(N, D).astype(np.float32))
    lab = labels.reshape(N).astype(np.int64)
    order = np.argsort(lab, kind="stable")
    e_s = e[order]
    lab_s = lab[order].astype(np.float32)
    labi = lab[order]

    # class windows: for sorted position g, w = min(class_start, N-128)
    starts = np.searchsorted(labi, labi, side="left")
    ends = np.searchsorted(labi, labi, side="right")
    assert int((ends - starts).max()) <= WMAX, "class larger than window"
    wof = np.minimum(starts, N - WMAX).astype(np.int64)

    efT = np.ascontiguousarray(e_s.T)                     # [D, N]
    labrow = lab_s.reshape(1, N)

    in_maps = []
    for r in range(NCORES):
        g0 = r * NLOC
        gg = np.arange(g0, g0 + NLOC)
        w = wof[gg]
        woff = (640 * np.arange(NLOC) + w).astype(np.int32).reshape(NLOC, 1)
        jpos = w[None, :] + np.arange(WMAX)[:, None]      # [WMAX, NLOC]
        posw = ((labi[jpos] == labi[gg][None, :])
                & (jpos != gg[None, :])).astype(np.float32)
        in_maps.append({
            "efT": efT,
            "elocT": np.ascontiguousarray(efT[:, g0:g0 + NLOC]),
            "labrow": labrow,
            "llocT": np.ascontiguousarray(lab_s[g0:g0 + NLOC].reshape(NLOC, 1)),
            "woff": woff,
            "posw": posw,
        })
    return in_maps


def run_on_device(embeddings: np.ndarray, labels: np.ndarray, **run_kwargs):
    from concourse.bass_utils import run_bass_kernel_spmd
    nc = _get_program()
    in_maps = _make_inputs(embeddings, labels)
    res = run_bass_kernel_spmd(nc, in_maps, core_ids=list(range(NCORES)),
                               **run_kwargs)
    total = 0.0
    count = 0.0
    for r in range(NCORES):
        o = res.results[r]["out"].astype(np.float64)
        tsum = res.results[r]["out2"].astype(np.float64).reshape(-1)
        dsum = o[0:NLOC, NT]
        for la in range(NLOC):
            t, s = la // 4, la % 4
            w = o[32 * s + 0, t] + o[32 * s + 1, t]
            p2 = o[32 * s + 2, t]
            q = o[32 * s + 3, t]
            if _is_act(la):   # sign anchor: M = (M' + 1)/2
                w = 0.5 * w + 0.5 * N * tsum[la]
                q = 0.5 * q + 0.5 * WMAX * N
                p2 = 0.5 * p2 + 0.5 * WMAX * dsum[la]
            total += w - p2
            count += q
    return total, count, res


def kernel(embeddings: np.ndarray, labels: np.ndarray):
    embeddings = np.asarray(embeddings)
    labels = np.asarray(labels)
    total, count, _ = run_on_device(embeddings, labels)

    lab = np.asarray(labels).reshape(-1)
    cnt = np.bincount(lab.astype(np.int64), minlength=1)
    per = cnt[lab.astype(np.int64)]
    num_valid = int(((per - 1) * (N - per)).sum())

    nv = np.float32(num_valid)
    ne = np.float32(count)
    tot = np.float32(total)
    if ne > 0:
        loss = np.float32(tot / np.maximum(ne, np.float32(1.0)))
    else:
        loss = np.float32(0.0)
    frac = np.float32(ne / (nv + np.float32(1e-16)))
    return (np.array(loss, np.float32), np.array(nv, np.float32),
            np.array(ne, np.float32), np.array(frac, np.float32))


# revision 16
# speedup vs baseline: 1.1459x; 1.1459x over previous
"""BatchAllTripletLoss on 8 Trainium2 NeuronCores via Bass/Tile.

Math: for anchors i, positives j (same label, j!=i), negatives k (diff label):
  total        = sum_{i,j,k} relu(d_ij - d_ik + margin)
  num_non_easy = #{(i,j,k): d_ik < d_ij + margin}
  loss         = total / num_non_easy ; frac = num_non_easy / num_valid

Key idea: samples are SORTED BY LABEL on the host, so each anchor's
positives j live in one contiguous window of at most 128 sorted positions.
The O(n^3) triplet work then needs only ONE [128, 640] compare tile per
anchor (j = its class window, k = all samples) instead of five:
  - masked row v'_k = d_ak + BIG*(same label), bf16, staged to DRAM and
    DMA-broadcast to [128, 4, 640] (stride-0 partition source, 4 anchors
    per transfer).
  - window thresholds t'_p = (d_{a, w_a+p} + margin) * posmask, gathered
    from the distance rows with ONE indirect DMA (per-anchor element
    offsets w_a are host data -> program is label-independent), f32,
    split hi/lo into bf16 lhsT columns [t_hi | t_lo | 1 | 1].
  - compare M[p, k] = (v'_k < t'_p): DVE is_lt bf16 (~300ns) for every
    other anchor, ACT Sign (host-corrected) for the rest.
  - PE reduces M: psum rows 32s..32s+3 (4 anchors per [128, 640] psum
    tile via explicit tile_position): t_hi*M, t_lo*M, q_k, q_k.
  - ONE drain per psum tile: DVE scalar_tensor_tensor against a weight
    tile holding [1 | 1 | dist row | 1] per anchor (+accum), giving the
    W rows, sum_k d_ak q_ak, and the count in one pass.
  total = sum(t'*M) - sum(d*q);  count = sum(q).  Host combines in f64.
num_valid is pure label counting (host, exact).
"""

import numpy as np

N = 640
D = 128
NCORES = 8
NLOC = N // NCORES            # 80 anchors per core
NT = NLOC // 4                # 20 psum tiles, 4 anchors each
MARGIN = 1.9
BIG = 1.0e9
WMAX = 128                    # class-window width (max class size)

_CACHE = {}


def _is_act(la):
    return la % 2 == 1


def _build_program():
    import concourse.bass as bass
    import concourse.bacc as bacc
    import concourse.mybir as mybir
    import concourse.tile as tile
    from concourse.masks import make_identity

    f32 = mybir.dt.float32
    bf16 = mybir.dt.bfloat16
    i32 = mybir.dt.int32
    Alu = mybir.AluOpType
    Act = mybir.ActivationFunctionType

    nc = bacc.Bacc("TRN2", target_bir_lowering=False, debug=False,
                   num_devices=NCORES)

    efT = nc.declare_dram_parameter("efT", [D, N], f32, isOutput=False)
    elocT = nc.declare_dram_parameter("elocT", [D, NLOC], f32, isOutput=False)
    labrow = nc.declare_dram_parameter("labrow", [1, N], f32, isOutput=False)
    llocT = nc.declare_dram_parameter("llocT", [NLOC, 1], f32, isOutput=False)
    woff = nc.declare_dram_parameter("woff", [NLOC, 1], i32, isOutput=False)
    posw = nc.declare_dram_parameter("posw", [WMAX, NLOC], f32, isOutput=False)
    # out cols: 0:NT fused stt sums (W rows / d*q row / count row), NT dsum
    OUTC = NT + 1
    out_d = nc.declare_dram_parameter("out", [128, OUTC], f32, isOutput=True)
    out2_d = nc.declare_dram_parameter("out2", [1, NLOC], f32, isOutput=True)

    from contextlib import ExitStack
    with tile.TileContext(nc) as tc:
        with (
            tc.tile_pool(name="singles", bufs=1) as sg,
            tc.tile_pool(name="vbp", bufs=5) as vbp,
            tc.tile_pool(name="mtp", bufs=10) as mtp,
            tc.tile_pool(name="drs", bufs=2) as drs,
            tc.tile_pool(name="dram", bufs=1, space="DRAM") as dram,
        ):
            pro_stack = ExitStack()
            ps_mm = pro_stack.enter_context(
                tc.tile_pool(name="ps_mm", bufs=1, space="PSUM"))
            ps_tr = pro_stack.enter_context(
                tc.tile_pool(name="ps_tr", bufs=1, space="PSUM"))

            # ---- load inputs (split across the two idle dispatch rings) ----
            EF = sg.tile([D, N], f32)
            nc.gpsimd.dma_start(out=EF[:, 0:320], in_=efT[:, 0:320])
            nc.sync.dma_start(out=EF[:, 320:N], in_=efT[:, 320:N])
            EL = sg.tile([D, NLOC], f32)
            nc.sync.dma_start(out=EL[:], in_=elocT[:])
            LR = sg.tile([1, N], f32)
            nc.sync.dma_start(out=LR[:], in_=labrow[:])
            LLT = sg.tile([NLOC, 1], f32)
            nc.sync.dma_start(out=LLT[:], in_=llocT[:])
            WOFF = sg.tile([NLOC, 1], i32)
            nc.sync.dma_start(out=WOFF[:], in_=woff[:])
            POSW = sg.tile([WMAX, NLOC], f32)
            nc.sync.dma_start(out=POSW[:], in_=posw[:])
            LBC = sg.tile([128, N], f32)
            nc.sync.dma_start(out=LBC[0:NLOC, :],
                              in_=labrow[:].to_broadcast([NLOC, N]))

            # weight tile for the fused drains: 1.0 everywhere except the
            # per-anchor dist rows (filled after DIST is computed)
            DPW = sg.tile([128, NT // 2, 2 * N], f32)
            nc.gpsimd.memset(DPW[:], 1.0)

            ident = sg.tile([128, 128], f32)
            make_identity(nc, ident[:])
            ones = sg.tile([128, 1], f32)
            nc.vector.memset(ones[:], 1.0)
            onesb = sg.tile([128, 1], bf16)
            nc.vector.memset(onesb[:], 1.0)
            onesrow = sg.tile([1, NLOC], bf16)
            nc.vector.memset(onesrow[:], 1.0)

            # ---- pairwise distance rows: d2 = sq_a + sq_k - 2 e_a.e_k ----
            # psum accumulates (-2 e_a.e_k) + broadcast sq_k; bf16 operands
            EFB = sg.tile([D, N], bf16)
            nc.vector.tensor_copy(EFB[:], EF[:])
            ELB = sg.tile([D, NLOC], bf16)
            nc.vector.tensor_scalar_mul(out=ELB[:], in0=EL[:], scalar1=-2.0)
            Esqb = sg.tile([D, N], bf16)
            nc.vector.tensor_mul(Esqb[:], EFB[:], EFB[:])
            ELsq = sg.tile([D, NLOC], f32)
            nc.vector.tensor_mul(ELsq[:], EL[:], EL[:])

            sqf_ps = ps_mm.tile([1, N], f32, tag="pro2", name="sqf")
            nc.tensor.matmul(sqf_ps[:, 0:512], onesb[:], Esqb[:, 0:512])
            nc.tensor.matmul(sqf_ps[:, 512:N], onesb[:], Esqb[:, 512:N])
            SQFB = sg.tile([1, N], bf16)
            nc.vector.tensor_copy(SQFB[:], sqf_ps[:])

            sql_ps = ps_tr.tile([NLOC, 1], f32, tag="pro3", name="sql")
            nc.tensor.matmul(sql_ps[:], ELsq[:], ones[:])
            SQL = sg.tile([NLOC, 1], f32)
            nc.vector.tensor_copy(SQL[:], sql_ps[:])

            dot_ps = ps_mm.tile([NLOC, N], f32, tag="pro", name="dot")
            nc.tensor.matmul(dot_ps[:, 0:512], ELB[:], EFB[:, 0:512],
                             start=True, stop=False)
            nc.tensor.matmul(dot_ps[:, 512:N], ELB[:], EFB[:, 512:N],
                             start=True, stop=False)
            nc.tensor.matmul(dot_ps[:, 0:512], onesrow[:],
                             SQFB[:, 0:512], start=False, stop=True,
                             skip_group_check=True)
            nc.tensor.matmul(dot_ps[:, 512:N], onesrow[:],
                             SQFB[:, 512:N], start=False, stop=True,
                             skip_group_check=True)

            PRE = sg.tile([NLOC, N], f32)
            nc.vector.tensor_scalar(out=PRE[:], in0=dot_ps[:], scalar1=SQL[:],
                                    scalar2=0.0, op0=Alu.add, op1=Alu.max)
            DIST = sg.tile([NLOC, N], f32)
            nc.scalar.activation(out=DIST[:], in_=PRE[:], func=Act.Sqrt)

            # masked v' rows, bf16 (fused add + cast)
            EQB = sg.tile([NLOC, N], f32)
            nc.vector.tensor_scalar(out=EQB[:], in0=LBC[0:NLOC, :], scalar1=LLT[:],
                                    scalar2=BIG, op0=Alu.is_equal, op1=Alu.mult)
            VMB = sg.tile([NLOC, N], bf16)
            nc.vector.tensor_add(VMB[:], DIST[:], EQB[:])
            vmd = dram.tile([NLOC, N], bf16)
            for ch in range(4):
                nc.sync.dma_start(out=vmd[20 * ch:20 * ch + 20, :],
                                  in_=VMB[20 * ch:20 * ch + 20, :])

            # dist rows -> DRAM (gather source), and into DPW rows 32s+2
            dist_d = dram.tile([1, NLOC * N], f32)
            dfl = dist_d[:]
            dst = bass.AP(tensor=dfl.tensor, offset=0,
                          ap=[[N, NLOC], [1, N]])
            nc.sync.dma_start(out=dst, in_=DIST[:])
            pstr = DPW[:].ap[0][0]
            for u in range(NT // 2):
                for h in range(2):
                    sl = DPW[2:3, u, h * N:h * N + N]
                    dpdst = bass.AP(tensor=sl.tensor, offset=sl.offset,
                                    ap=[[32 * pstr, 4], [1, N]])
                    nc.sync.dma_start(out=dpdst,
                                      in_=DIST[8 * u + 4 * h:8 * u + 4 * h + 4, :])

            # window thresholds via indirect gather: TQR[la, p] =
            # dist[la, w_la + p]; offsets woff = 640*la + w_la
            TQR = sg.tile([NLOC, WMAX], f32)
            nc.gpsimd.indirect_dma_start(
                out=TQR[:], out_offset=None,
                in_=dist_d[:],
                in_offset=bass.IndirectOffsetOnAxis(ap=WOFF[:, :1], axis=1),
            )
            tq_ps = ps_tr.tile([WMAX, NLOC], f32, tag="tr", name="tq")
            nc.tensor.transpose(tq_ps[:], TQR[:], ident[0:NLOC, 0:NLOC])
            TQ = sg.tile([WMAX, NLOC], f32)
            nc.vector.tensor_scalar_add(out=TQ[:], in0=tq_ps[:], scalar1=MARGIN)
            nc.vector.tensor_mul(TQ[:], TQ[:], POSW[:])

            # lhsT tiles [128, NLOC, 4] bf16: [t_hi | t_lo | 1 | 1]
            LHSB = sg.tile([WMAX, NLOC, 4], bf16)
            nc.vector.tensor_copy(LHSB[:, :, 0], TQ[:])
            thf = sg.tile([WMAX, NLOC], f32)
            nc.vector.tensor_copy(thf[:], LHSB[:, :, 0])
            nc.vector.tensor_sub(thf[:], TQ[:], thf[:])
            nc.vector.tensor_copy(LHSB[:, :, 1], thf[:])
            nc.vector.memset(LHSB[:, :, 2:4], 1.0)

            # Tsum_a = sum_p t'_ap (for ACT-sign corrections)
            ts_ps = ps_tr.tile([1, NLOC], f32, tag="tr", name="ts_ps")
            nc.tensor.matmul(ts_ps[:], ones[:], TQ[:])
            TSROW = sg.tile([1, NLOC], f32)
            nc.vector.tensor_copy(TSROW[:], ts_ps[:])
            nc.sync.dma_start(out=out2_d[:], in_=TSROW[:])

            # dist row sums (for sign-anchor corrections)
            DSC = sg.tile([NLOC, N], f32)
            DSUM = sg.tile([NLOC, 1], f32)
            nc.scalar.activation(out=DSC[:], in_=DIST[:], func=Act.Identity,
                                 bias=0.0, scale=1.0, accum_out=DSUM[:])

            pro_stack.close()
            wq_stack = ExitStack()
            ps_wq = wq_stack.enter_context(
                tc.tile_pool(name="ps_wq", bufs=3, space="PSUM"))

            SC = sg.tile([128, NT], f32)

            # ---- main loop: one [128, 640] compare + matmul pair/anchor ----
            vb8_cache = {}
            wq = None
            for la in range(NLOC):
                u, h, s = la // 8, (la // 4) % 2, la % 4
                if la % 8 == 0:
                    vb8 = vbp.tile([128, 8, N], bf16, tag="vb", name="vb")
                    sl = vmd[la:la + 8, :]
                    bsrc = bass.AP(tensor=sl.tensor, offset=sl.offset,
                                   ap=[[0, 128]] + [list(p) for p in sl.ap])
                    nc.gpsimd.dma_start(out=vb8[:], in_=bsrc)
                    vb8_cache[0] = vb8
                if s == 0:
                    wq = ps_wq.tile([128, N], f32, tag="wq", name="wq")
                vb = vb8_cache[0][:, la % 8, :]
                mt = mtp.tile([128, N], bf16, tag="mt", name="mt")
                if _is_act(la):
                    nc.scalar.activation(out=mt[:], in_=vb[:], func=Act.Sign,
                                         bias=TQ[:, la:la + 1], scale=-1.0)
                else:
                    nc.vector.tensor_scalar(out=mt[:], in0=vb[:],
                                            scalar1=TQ[:, la:la + 1],
                                            scalar2=None, op0=Alu.is_lt)
                nc.tensor.matmul(wq[32 * s:32 * s + 4, 0:512],
                                 LHSB[:, la], mt[:, 0:512],
                                 start=True, stop=True,
                                 tile_position=(0, 32 * s))
                nc.tensor.matmul(wq[32 * s:32 * s + 4, 512:N],
                                 LHSB[:, la], mt[:, 512:N],
                                 start=True, stop=True,
                                 tile_position=(0, 32 * s))
                if s == 3:
                    t = la // 4
                    sb = drs.tile([128, N], bf16, tag="sb", name="sb")
                    nc.vector.scalar_tensor_tensor(out=sb[:], in0=wq[:],
                                                   scalar=1.0,
                                                   in1=DPW[:, t // 2,
                                                           (t % 2) * N:
                                                           (t % 2) * N + N],
                                                   op0=Alu.mult, op1=Alu.mult,
                                                   accum_out=SC[:, t:t + 1])

            # ---- stage outputs ----
            OUTS = sg.tile([128, OUTC], f32)
            nc.vector.tensor_copy(OUTS[:, 0:NT], SC[:])
            nc.vector.tensor_copy(OUTS[0:NLOC, NT:NT + 1], DSUM[:])
            nc.gpsimd.dma_start(out=out_d[:], in_=OUTS[:])
            wq_stack.close()

    nc.compile()
    return nc


def _get_program():
    if "nc" not in _CACHE:
        _CACHE["nc"] = _build_program()
    return _CACHE["nc"]


def _make_inputs(embeddings: np.ndarray, labels: np.ndarray):
    e = np.ascontiguousarray(embeddings.reshape(N, D).astype(np.float32))
    lab = labels.reshape(N).astype(np.int64)
    order = np.argsort(lab, kind="stable")
    e_s = e[order]
    lab_s = lab[order].astype(np.float32)
    labi = lab[order]

    # class windows: for sorted position g, w = min(class_start, N-128)
    starts = np.searchsorted(labi, labi, side="left")
    ends = np.searchsorted(labi, labi, side="right")
    assert int((ends - starts).max()) <= WMAX, "class larger than window"
    wof = np.minimum(starts, N - WMAX).astype(np.int64)

    efT = np.ascontiguousarray(e_s.T)                     # [D, N]
    labrow = lab_s.reshape(1, N)

    in_maps = []
    for r in range(NCORES):
        g0 = r * NLOC
        gg = np.arange(g0, g0 + NLOC)
        w = wof[gg]
        woff = (640 * np.arange(NLOC) + w).astype(np.int32).reshape(NLOC, 1)
        jpos = w[None, :] + np.arange(WMAX)[:, None]      # [WMAX, NLOC]
        posw = ((labi[jpos] == labi[gg][None, :])
                & (jpos != gg[None, :])).astype(np.float32)
        in_maps.append({
            "efT": efT,
            "elocT": np.ascontiguousarray(efT[:, g0:g0 + NLOC]),
            "labrow": labrow,
            "llocT": np.ascontiguousarray(lab_s[g0:g0 + NLOC].reshape(NLOC, 1)),
            "woff": woff,
            "posw": posw,
        })
    return in_maps


def run_on_device(embeddings: np.ndarray, labels: np.ndarray, **run_kwargs):
    from concourse.bass_utils import run_bass_kernel_spmd
    nc = _get_program()
    in_maps = _make_inputs(embeddings, labels)
    res = run_bass_kernel_spmd(nc, in_maps, core_ids=list(range(NCORES)),
                               **run_kwargs)
    total = 0.0
    count = 0.0
    for r in range(NCORES):
        o = res.results[r]["out"].astype(np.float64)
        tsum = res.results[r]["out2"].astype(np.float64).reshape(-1)
        dsum = o[0:NLOC, NT]
        for la in range(NLOC):
            t, s = la // 4, la % 4
            w = o[32 * s + 0, t] + o[32 * s + 1, t]
            p2 = o[32 * s + 2, t]
            q = o[32 * s + 3, t]
            if _is_act(la):   # sign anchor: M = (M' + 1)/2
                w = 0.5 * w + 0.5 * N * tsum[la]
                q = 0.5 * q + 0.5 * WMAX * N
                p2 = 0.5 * p2 + 0.5 * WMAX * dsum[la]
            total += w - p2
            count += q
    return total, count, res


def kernel(embeddings: np.ndarray, labels: np.ndarray):
    embeddings = np.asarray(embeddings)
    labels = np.asarray(labels)
    total, count, _ = run_on_device(embeddings, labels)

    lab = np.asarray(labels).reshape(-1)
    cnt = np.bincount(lab.astype(np.int64), minlength=1)
    per = cnt[lab.astype(np.int64)]
    num_valid = int(((per - 1) * (N - per)).sum())

    nv = np.float32(num_valid)
    ne = np.float32(count)
    tot = np.float32(total)
    if ne > 0:
        loss = np.float32(tot / np.maximum(ne, np.float32(1.0)))
    else:
        loss = np.float32(0.0)
    frac = np.float32(ne / (nv + np.float32(1e-16)))
    return (np.array(loss, np.float32), np.array(nv, np.float32),
            np.array(ne, np.float32), np.array(frac, np.float32))


# revision 17
# speedup vs baseline: 1.1471x; 1.0010x over previous
"""BatchAllTripletLoss on 8 Trainium2 NeuronCores via Bass/Tile.

Math: for anchors i, positives j (same label, j!=i), negatives k (diff label):
  total        = sum_{i,j,k} relu(d_ij - d_ik + margin)
  num_non_easy = #{(i,j,k): d_ik < d_ij + margin}
  loss         = total / num_non_easy ; frac = num_non_easy / num_valid

Key idea: samples are SORTED BY LABEL on the host, so each anchor's
positives j live in one contiguous window of at most 128 sorted positions.
The O(n^3) triplet work then needs only ONE [128, 640] compare tile per
anchor (j = its class window, k = all samples) instead of five:
  - masked row v'_k = d_ak + BIG*(same label), bf16, staged to DRAM and
    DMA-broadcast to [128, 4, 640] (stride-0 partition source, 4 anchors
    per transfer).
  - window thresholds t'_p = (d_{a, w_a+p} + margin) * posmask, gathered
    from the distance rows with ONE indirect DMA (per-anchor element
    offsets w_a are host data -> program is label-independent), f32,
    split hi/lo into bf16 lhsT columns [t_hi | t_lo | 1 | 1].
  - compare M[p, k] = (v'_k < t'_p): DVE is_lt bf16 (~300ns) for every
    other anchor, ACT Sign (host-corrected) for the rest.
  - PE reduces M: psum rows 32s..32s+3 (4 anchors per [128, 640] psum
    tile via explicit tile_position): t_hi*M, t_lo*M, q_k, q_k.
  - ONE drain per psum tile: DVE scalar_tensor_tensor against a weight
    tile holding [1 | 1 | dist row | 1] per anchor (+accum), giving the
    W rows, sum_k d_ak q_ak, and the count in one pass.
  total = sum(t'*M) - sum(d*q);  count = sum(q).  Host combines in f64.
num_valid is pure label counting (host, exact).
"""

import numpy as np

N = 640
D = 128
NCORES = 8
NLOC = N // NCORES            # 80 anchors per core
NT = NLOC // 4                # 20 psum tiles, 4 anchors each
MARGIN = 1.9
BIG = 1.0e9
WMAX = 128                    # class-window width (max class size)

_CACHE = {}


def _is_act(la):
    return la % 2 == 1


def _build_program():
    import concourse.bass as bass
    import concourse.bacc as bacc
    import concourse.mybir as mybir
    import concourse.tile as tile
    from concourse.masks import make_identity

    f32 = mybir.dt.float32
    bf16 = mybir.dt.bfloat16
    i32 = mybir.dt.int32
    Alu = mybir.AluOpType
    Act = mybir.ActivationFunctionType

    nc = bacc.Bacc("TRN2", target_bir_lowering=False, debug=False,
                   num_devices=NCORES)

    efT = nc.declare_dram_parameter("efT", [D, N], f32, isOutput=False)
    elocT = nc.declare_dram_parameter("elocT", [D, NLOC], f32, isOutput=False)
    labrow = nc.declare_dram_parameter("labrow", [1, N], f32, isOutput=False)
    llocT = nc.declare_dram_parameter("llocT", [NLOC, 1], f32, isOutput=False)
    woff = nc.declare_dram_parameter("woff", [NLOC, 1], i32, isOutput=False)
    posw = nc.declare_dram_parameter("posw", [WMAX, NLOC], f32, isOutput=False)
    # out cols: 0:NT fused stt sums (W rows / d*q row / count row), NT dsum
    OUTC = NT + 1
    out_d = nc.declare_dram_parameter("out", [128, OUTC], f32, isOutput=True)
    out2_d = nc.declare_dram_parameter("out2", [1, NLOC], f32, isOutput=True)

    from contextlib import ExitStack
    with tile.TileContext(nc) as tc:
        with (
            tc.tile_pool(name="singles", bufs=1) as sg,
            tc.tile_pool(name="vbp", bufs=5) as vbp,
            tc.tile_pool(name="mtp", bufs=10) as mtp,
            tc.tile_pool(name="drs", bufs=2) as drs,
            tc.tile_pool(name="dram", bufs=1, space="DRAM") as dram,
        ):
            pro_stack = ExitStack()
            ps_mm = pro_stack.enter_context(
                tc.tile_pool(name="ps_mm", bufs=1, space="PSUM"))
            ps_tr = pro_stack.enter_context(
                tc.tile_pool(name="ps_tr", bufs=1, space="PSUM"))

            # ---- load inputs (split across the two idle dispatch rings) ----
            EF = sg.tile([D, N], f32)
            nc.gpsimd.dma_start(out=EF[:, 0:320], in_=efT[:, 0:320])
            nc.sync.dma_start(out=EF[:, 320:N], in_=efT[:, 320:N])
            EL = sg.tile([D, NLOC], f32)
            nc.sync.dma_start(out=EL[:], in_=elocT[:])
            LR = sg.tile([1, N], f32)
            nc.sync.dma_start(out=LR[:], in_=labrow[:])
            LLT = sg.tile([NLOC, 1], f32)
            nc.sync.dma_start(out=LLT[:], in_=llocT[:])
            WOFF = sg.tile([NLOC, 1], i32)
            nc.sync.dma_start(out=WOFF[:], in_=woff[:])
            POSW = sg.tile([WMAX, NLOC], f32)
            nc.sync.dma_start(out=POSW[:], in_=posw[:])
            LBC = sg.tile([128, N], f32)
            nc.sync.dma_start(out=LBC[0:NLOC, :],
                              in_=labrow[:].to_broadcast([NLOC, N]))

            # ping-pong weight tiles for the fused drains: 1.0 everywhere
            # except the per-anchor dist rows (rewritten per psum tile)
            DPW2 = []
            for i in range(2):
                d = sg.tile([128, N], f32, tag=f"dpw{i}", name=f"dpw{i}")
                nc.vector.memset(d[:], 1.0)
                DPW2.append(d)

            ident = sg.tile([128, 128], f32)
            make_identity(nc, ident[:])
            ones = sg.tile([128, 1], f32)
            nc.vector.memset(ones[:], 1.0)
            onesb = sg.tile([128, 1], bf16)
            nc.vector.memset(onesb[:], 1.0)
            onesrow = sg.tile([1, NLOC], bf16)
            nc.vector.memset(onesrow[:], 1.0)

            # ---- pairwise distance rows: d2 = sq_a + sq_k - 2 e_a.e_k ----
            # psum accumulates (-2 e_a.e_k) + broadcast sq_k; bf16 operands
            EFB = sg.tile([D, N], bf16)
            nc.vector.tensor_copy(EFB[:], EF[:])
            ELB = sg.tile([D, NLOC], bf16)
            nc.vector.tensor_scalar_mul(out=ELB[:], in0=EL[:], scalar1=-2.0)
            Esqb = sg.tile([D, N], bf16)
            nc.vector.tensor_mul(Esqb[:], EFB[:], EFB[:])
            ELsq = sg.tile([D, NLOC], f32)
            nc.vector.tensor_mul(ELsq[:], EL[:], EL[:])

            sqf_ps = ps_mm.tile([1, N], f32, tag="pro2", name="sqf")
            nc.tensor.matmul(sqf_ps[:, 0:512], onesb[:], Esqb[:, 0:512])
            nc.tensor.matmul(sqf_ps[:, 512:N], onesb[:], Esqb[:, 512:N])
            SQFB = sg.tile([1, N], bf16)
            nc.vector.tensor_copy(SQFB[:], sqf_ps[:])

            sql_ps = ps_tr.tile([NLOC, 1], f32, tag="pro3", name="sql")
            nc.tensor.matmul(sql_ps[:], ELsq[:], ones[:])
            SQL = sg.tile([NLOC, 1], f32)
            nc.vector.tensor_copy(SQL[:], sql_ps[:])

            dot_ps = ps_mm.tile([NLOC, N], f32, tag="pro", name="dot")
            nc.tensor.matmul(dot_ps[:, 0:512], ELB[:], EFB[:, 0:512],
                             start=True, stop=False)
            nc.tensor.matmul(dot_ps[:, 512:N], ELB[:], EFB[:, 512:N],
                             start=True, stop=False)
            nc.tensor.matmul(dot_ps[:, 0:512], onesrow[:],
                             SQFB[:, 0:512], start=False, stop=True,
                             skip_group_check=True)
            nc.tensor.matmul(dot_ps[:, 512:N], onesrow[:],
                             SQFB[:, 512:N], start=False, stop=True,
                             skip_group_check=True)

            PRE = sg.tile([NLOC, N], f32)
            nc.vector.tensor_scalar(out=PRE[:], in0=dot_ps[:], scalar1=SQL[:],
                                    scalar2=0.0, op0=Alu.add, op1=Alu.max)
            DIST = sg.tile([NLOC, N], f32)
            nc.scalar.activation(out=DIST[:], in_=PRE[:], func=Act.Sqrt)

            # masked v' rows, bf16 (fused add + cast)
            EQB = sg.tile([NLOC, N], f32)
            nc.vector.tensor_scalar(out=EQB[:], in0=LBC[0:NLOC, :], scalar1=LLT[:],
                                    scalar2=BIG, op0=Alu.is_equal, op1=Alu.mult)
            VMB = sg.tile([NLOC, N], bf16)
            nc.vector.tensor_add(VMB[:], DIST[:], EQB[:])
            vmd = dram.tile([NLOC, N], bf16)
            for ch in range(4):
                nc.sync.dma_start(out=vmd[20 * ch:20 * ch + 20, :],
                                  in_=VMB[20 * ch:20 * ch + 20, :])

            # dist rows -> DRAM (gather source), and into DPW rows 32s+2
            dist_d = dram.tile([1, NLOC * N], f32)
            dfl = dist_d[:]
            dst = bass.AP(tensor=dfl.tensor, offset=0,
                          ap=[[N, NLOC], [1, N]])
            nc.sync.dma_start(out=dst, in_=DIST[:])
            pstr = DPW2[0][:].ap[0][0]

            # window thresholds via indirect gather: TQR[la, p] =
            # dist[la, w_la + p]; offsets woff = 640*la + w_la
            TQR = sg.tile([NLOC, WMAX], f32)
            nc.gpsimd.indirect_dma_start(
                out=TQR[:], out_offset=None,
                in_=dist_d[:],
                in_offset=bass.IndirectOffsetOnAxis(ap=WOFF[:, :1], axis=1),
            )
            tq_ps = ps_tr.tile([WMAX, NLOC], f32, tag="tr", name="tq")
            nc.tensor.transpose(tq_ps[:], TQR[:], ident[0:NLOC, 0:NLOC])
            TQ = sg.tile([WMAX, NLOC], f32)
            nc.vector.tensor_scalar_add(out=TQ[:], in0=tq_ps[:], scalar1=MARGIN)
            nc.vector.tensor_mul(TQ[:], TQ[:], POSW[:])

            # lhsT tiles [128, NLOC, 4] bf16: [t_hi | t_lo | 1 | 1]
            LHSB = sg.tile([WMAX, NLOC, 4], bf16)
            nc.vector.tensor_copy(LHSB[:, :, 0], TQ[:])
            thf = sg.tile([WMAX, NLOC], f32)
            nc.vector.tensor_copy(thf[:], LHSB[:, :, 0])
            nc.vector.tensor_sub(thf[:], TQ[:], thf[:])
            nc.vector.tensor_copy(LHSB[:, :, 1], thf[:])
            nc.vector.memset(LHSB[:, :, 2:4], 1.0)

            # Tsum_a = sum_p t'_ap (for ACT-sign corrections)
            ts_ps = ps_tr.tile([1, NLOC], f32, tag="tr", name="ts_ps")
            nc.tensor.matmul(ts_ps[:], ones[:], TQ[:])
            TSROW = sg.tile([1, NLOC], f32)
            nc.vector.tensor_copy(TSROW[:], ts_ps[:])
            nc.sync.dma_start(out=out2_d[:], in_=TSROW[:])

            # dist row sums (for sign-anchor corrections)
            DSC = sg.tile([NLOC, N], f32)
            DSUM = sg.tile([NLOC, 1], f32)
            nc.scalar.activation(out=DSC[:], in_=DIST[:], func=Act.Identity,
                                 bias=0.0, scale=1.0, accum_out=DSUM[:])

            pro_stack.close()
            wq_stack = ExitStack()
            ps_wq = wq_stack.enter_context(
                tc.tile_pool(name="ps_wq", bufs=3, space="PSUM"))

            SC = sg.tile([128, NT], f32)

            # ---- main loop: one [128, 640] compare + matmul pair/anchor ----
            vb8_cache = {}
            wq = None
            for la in range(NLOC):
                u, h, s = la // 8, (la // 4) % 2, la % 4
                if la % 8 == 0:
                    vb8 = vbp.tile([128, 8, N], bf16, tag="vb", name="vb")
                    sl = vmd[la:la + 8, :]
                    bsrc = bass.AP(tensor=sl.tensor, offset=sl.offset,
                                   ap=[[0, 128]] + [list(p) for p in sl.ap])
                    nc.gpsimd.dma_start(out=vb8[:], in_=bsrc)
                    vb8_cache[0] = vb8
                if s == 0:
                    wq = ps_wq.tile([128, N], f32, tag="wq", name="wq")
                    tt = la // 4
                    sl = DPW2[tt % 2][2:3, :]
                    dpdst = bass.AP(tensor=sl.tensor, offset=sl.offset,
                                    ap=[[32 * pstr, 4], [1, N]])
                    nc.sync.dma_start(out=dpdst,
                                      in_=DIST[4 * tt:4 * tt + 4, :])
                vb = vb8_cache[0][:, la % 8, :]
                mt = mtp.tile([128, N], bf16, tag="mt", name="mt")
                if _is_act(la):
                    nc.scalar.activation(out=mt[:], in_=vb[:], func=Act.Sign,
                                         bias=TQ[:, la:la + 1], scale=-1.0)
                else:
                    nc.vector.tensor_scalar(out=mt[:], in0=vb[:],
                                            scalar1=TQ[:, la:la + 1],
                                            scalar2=None, op0=Alu.is_lt)
                nc.tensor.matmul(wq[32 * s:32 * s + 4, 0:512],
                                 LHSB[:, la], mt[:, 0:512],
                                 start=True, stop=True,
                                 tile_position=(0, 32 * s))
                nc.tensor.matmul(wq[32 * s:32 * s + 4, 512:N],
                                 LHSB[:, la], mt[:, 512:N],
                                 start=True, stop=True,
                                 tile_position=(0, 32 * s))
                if s == 3:
                    t = la // 4
                    sb = drs.tile([128, N], bf16, tag="sb", name="sb")
                    nc.vector.scalar_tensor_tensor(out=sb[:], in0=wq[:],
                                                   scalar=1.0,
                                                   in1=DPW2[t % 2][:],
                                                   op0=Alu.mult, op1=Alu.mult,
                                                   accum_out=SC[:, t:t + 1])

            # ---- stage outputs ----
            OUTS = sg.tile([128, OUTC], f32)
            nc.vector.tensor_copy(OUTS[:, 0:NT], SC[:])
            nc.vector.tensor_copy(OUTS[0:NLOC, NT:NT + 1], DSUM[:])
            nc.gpsimd.dma_start(out=out_d[:], in_=OUTS[:])
            wq_stack.close()

    nc.compile()
    return nc


def _get_program():
    if "nc" not in _CACHE:
        _CACHE["nc"] = _build_program()
    return _CACHE["nc"]


def _make_inputs(embeddings: np.ndarray, labels: np.ndarray):
    e = np.ascontiguousarray(embeddings.reshape(N, D).astype(np.float32))
    lab = labels.reshape(N).astype(np.int64)
    order = np.argsort(lab, kind="stable")
    e_s = e[order]
    lab_s = lab[order].astype(np.float32)
    labi = lab[order]

    # class windows: for sorted position g, w = min(class_start, N-128)
    starts = np.searchsorted(labi, labi, side="left")
    ends = np.searchsorted(labi, labi, side="right")
    assert int((ends - starts).max()) <= WMAX, "class larger than window"
    wof = np.minimum(starts, N - WMAX).astype(np.int64)

    efT = np.ascontiguousarray(e_s.T)                     # [D, N]
    labrow = lab_s.reshape(1, N)

    in_maps = []
    for r in range(NCORES):
        g0 = r * NLOC
        gg = np.arange(g0, g0 + NLOC)
        w = wof[gg]
        woff = (640 * np.arange(NLOC) + w).astype(np.int32).reshape(NLOC, 1)
        jpos = w[None, :] + np.arange(WMAX)[:, None]      # [WMAX, NLOC]
        posw = ((labi[jpos] == labi[gg][None, :])
                & (jpos != gg[None, :])).astype(np.float32)
        in_maps.append({
            "efT": efT,
            "elocT": np.ascontiguousarray(efT[:, g0:g0 + NLOC]),
            "labrow": labrow,
            "llocT": np.ascontiguousarray(lab_s[g0:g0 + NLOC].reshape(NLOC, 1)),
            "woff": woff,
            "posw": posw,
        })
    return in_maps


def run_on_device(embeddings: np.ndarray, labels: np.ndarray, **run_kwargs):
    from concourse.bass_utils import run_bass_kernel_spmd
    nc = _get_program()
    in_maps = _make_inputs(embeddings, labels)
    res = run_bass_kernel_spmd(nc, in_maps, core_ids=list(range(NCORES)),
                               **run_kwargs)
    total = 0.0
    count = 0.0
    for r in range(NCORES):
        o = res.results[r]["out"].astype(np.float64)
        tsum = res.results[r]["out2"].astype(np.float64).reshape(-1)
        dsum = o[0:NLOC, NT]
        for la in range(NLOC):
            t, s = la // 4, la % 4
            w = o[32 * s + 0, t] + o[32 * s + 1, t]
            p2 = o[32 * s + 2, t]
            q = o[32 * s + 3, t]
            if _is_act(la):   # sign anchor: M = (M' + 1)/2
                w = 0.5 * w + 0.5 * N * tsum[la]
                q = 0.5 * q + 0.5 * WMAX * N
                p2 = 0.5 * p2 + 0.5 * WMAX * dsum[la]
            total += w - p2
            count += q
    return total, count, res


def kernel(embeddings: np.ndarray, labels: np.ndarray):
    embeddings = np.asarray(embeddings)
    labels = np.asarray(labels)
    total, count, _ = run_on_device(embeddings, labels)

    lab = np.asarray(labels).reshape(-1)
    cnt = np.bincount(lab.astype(np.int64), minlength=1)
    per = cnt[lab.astype(np.int64)]
    num_valid = int(((per - 1) * (N - per)).sum())

    nv = np.float32(num_valid)
    ne = np.float32(count)
    tot = np.float32(total)
    if ne > 0:
        loss = np.float32(tot / np.maximum(ne, np.float32(1.0)))
    else:
        loss = np.float32(0.0)
    frac = np.float32(ne / (nv + np.float32(1e-16)))
    return (np.array(loss, np.float32), np.array(nv, np.float32),
            np.array(ne, np.float32), np.array(frac, np.float32))
